# revision 1
# baseline (speedup 1.0000x reference)
"""Inverse Hough transform (nn_C_iht) on 8 Trainium2 NeuronCores.

out[n,c,y,x] = sum_a hough[n,c,a, r(a,y,x)]  with a static index table r.

Strategy (per core; batch n is sharded across the 8 cores, c=128 channels sit
on the SBUF partition axis):
  - The gather-sum is evaluated as a sequence of one-hot matmuls on the
    TensorEngine.  For a pixel block P (bw x bh = 128 pixels) and a chunk
    C = (16 consecutive angles) x (8 consecutive rhos), K = 128:
        psum[c, px] += Hp_chunk[k, c].T  @  E_chunk[k, px]
    where Hp_chunk is an affine slice of a host-side rectangle re-layout of
    the input and E_chunk in {0,1} is the (static, precomputed) one-hot
    selector  E[(ai,rj), px] = [ r(a, px) == rho ].
  - E is streamed from HBM as fp8e4 (0/1 exact); H stays bf16 (mixed-dtype
    matmul, fp32 PSUM accumulation).
"""

import sys

sys.path.insert(0, "/opt/trn_rl_repo")

import numpy as np
import ml_dtypes

N, C, HIMG, WIMG = 8, 128, 160, 160
NUMANGLE, NUMRHO = 180, 180

# chunk geometry
G = 16         # angles per chunk
B = 8          # rhos per chunk  (G * B = 128 = contraction dim)
APAD = 192     # padded angle count  (12 groups of 16)
RPAD = 192     # padded rho count    (24 rho-blocks of 8)
NG = APAD // G          # 12 angle groups
NR = RPAD // B          # 24 rho blocks
BW, BH = 8, 16          # pixel block: 8 wide (x), 16 tall (y) -> 128 px
NBX, NBY = WIMG // BW, HIMG // BH   # 20 x 10 = 200 blocks
XGRP = 4                # blocks per output-staging DMA (adjacent in x)

F8 = ml_dtypes.float8_e4m3
BF16 = ml_dtypes.bfloat16


def _rho_table() -> np.ndarray:
    """Exact replica of the reference's index table r[a, y, x]."""
    irho = (int(np.sqrt(HIMG * HIMG + WIMG * WIMG)) + 1) / float(NUMRHO)
    itheta = np.pi / NUMANGLE
    theta = np.arange(NUMANGLE) * itheta
    tab_cos = np.cos(theta) / irho
    tab_sin = np.sin(theta) / irho
    xs = np.arange(WIMG) - WIMG // 2
    ys = np.arange(HIMG) - HIMG // 2
    r = np.round(xs[None, None, :] * tab_cos[:, None, None]
                 + ys[None, :, None] * tab_sin[:, None, None]).astype(np.int64)
    return np.clip(r + NUMRHO // 2, 0, NUMRHO - 1)  # [A, H, W]


def _build_schedule():
    """Static chunk schedule + packed one-hot E stream.

    Returns (chunks_per_block, e_stream) where chunks_per_block is a list
    (block-major order: by, then bx) of lists of (g, r) and e_stream is the
    concatenated fp8 E payload, one [128, 128] chunk per schedule entry.
    """
    R = _rho_table()
    blocks = []
    e_parts = []
    for by in range(NBY):
        for bx in range(NBX):
            sub = R[:, by * BH:(by + 1) * BH, bx * BW:(bx + 1) * BW]
            sub = sub.reshape(NUMANGLE, BH * BW)  # px = dy*BW + dx
            chunk_list = []
            for g in range(NG):
                a0, a1 = g * G, min((g + 1) * G, NUMANGLE)
                if a0 >= NUMANGLE:
                    break
                asub = sub[a0:a1]                      # [na, 128]
                lo, hi = int(asub.min()), int(asub.max())
                for r in range(lo // B, hi // B + 1):
                    ks = asub - r * B                  # rho_j per (ai, px)
                    valid = (ks >= 0) & (ks < B)
                    if not valid.any():
                        continue
                    e = np.zeros((G * B, BH * BW), np.float32)
                    ai, px = np.nonzero(valid)
                    e[ai * B + ks[ai, px], px] = 1.0
                    chunk_list.append((g, r))
                    e_parts.append(e.astype(F8))
            blocks.append(chunk_list)
    e_stream = np.concatenate(e_parts, axis=1) if e_parts else np.zeros((128, 0), F8)
    return blocks, np.ascontiguousarray(e_stream)


def _pack_h(h_core: np.ndarray) -> np.ndarray:
    """[C, A, RHO] fp32 -> rectangle layout [128, NG*NR*128] bf16.

    Hp[ai*B+rj, ((g*NR)+r)*128 + c] = h[c, g*G+ai, r*B+rj]
    """
    hp = np.zeros((C, APAD, RPAD), np.float32)
    hp[:, :NUMANGLE, :NUMRHO] = h_core
    hp = hp.reshape(C, NG, G, NR, B)
    hp = hp.transpose(2, 4, 1, 3, 0)           # [G, B, NG, NR, C]
    return np.ascontiguousarray(hp.reshape(G * B, NG * NR * C).astype(BF16))


_SCHED_CACHE = None


def _schedule():
    global _SCHED_CACHE
    if _SCHED_CACHE is None:
        _SCHED_CACHE = _build_schedule()
    return _SCHED_CACHE


def build_bass(reps: int = 1, nby_limit: int | None = None):
    """Build the Bass program (single-core SPMD; same program on all cores)."""
    import concourse.mybir as mybir
    from concourse import bacc
    from concourse.tile import TileContext

    blocks, e_stream = _schedule()
    total_chunks = sum(len(b) for b in blocks)
    assert e_stream.shape == (128, total_chunks * 128)

    nc = bacc.Bacc(None, target_bir_lowering=False)
    hp_d = nc.dram_tensor("hp", [128, NG * NR * C], mybir.dt.bfloat16,
                          kind="ExternalInput")
    e_d = nc.dram_tensor("e", [128, total_chunks * 128], mybir.dt.float8e4,
                         kind="ExternalInput")
    out_d = nc.dram_tensor("out", [128, HIMG * WIMG], mybir.dt.float32,
                           kind="ExternalOutput")

    max_ch = max(len(b) for b in blocks)

    with TileContext(nc) as tc:
        with tc.tile_pool(name="hp_pool", bufs=1) as hp_pool, \
             tc.tile_pool(name="e_pool", bufs=3) as e_pool, \
             tc.tile_pool(name="stage_pool", bufs=3) as stage_pool, \
             tc.tile_pool(name="psum_pool", bufs=8, space="PSUM") as psum_pool:
            hp_t = hp_pool.tile([128, NG * NR * C], mybir.dt.bfloat16)
            nc.sync.dma_start(hp_t[:], hp_d[:])

            nby_run = NBY if nby_limit is None else nby_limit
            for _ in range(reps):
                e_off = 0
                bi = 0
                for by in range(nby_run):
                    for bxg in range(NBX // XGRP):
                        stage = stage_pool.tile([128, XGRP * BW * BH],
                                                mybir.dt.float32, tag="stage")
                        for bxi in range(XGRP):
                            chunk_list = blocks[bi]
                            nch = len(chunk_list)
                            et = e_pool.tile([128, max_ch * 128],
                                             mybir.dt.float8e4, tag="e")
                            nc.sync.dma_start(
                                et[:, :nch * 128],
                                e_d[:, e_off * 128:(e_off + nch) * 128])
                            ps = psum_pool.tile([128, BW * BH],
                                                mybir.dt.float32, tag="ps")
                            for ci, (g, r) in enumerate(chunk_list):
                                col = (g * NR + r) * C
                                nc.tensor.matmul(
                                    ps[:],
                                    hp_t[:, col:col + C],
                                    et[:, ci * 128:(ci + 1) * 128],
                                    start=(ci == 0),
                                    stop=(ci == nch - 1),
                                )
                            # drain psum -> stage (px = dy*BW+dx -> dy*(XGRP*BW) + bxi*BW + dx)
                            nc.vector.tensor_copy(
                                stage[:].rearrange(
                                    "p (dy bxs dx) -> p dy bxs dx",
                                    dy=BH, bxs=XGRP)[:, :, bxi, :],
                                ps[:].rearrange("p (dy dx) -> p dy dx", dy=BH),
                            )
                            e_off += nch
                            bi += 1
                        bx0 = bxg * XGRP
                        dst = out_d[:].rearrange("p (y x) -> p y x", y=HIMG)
                        nc.sync.dma_start(
                            dst[:, by * BH:(by + 1) * BH,
                                bx0 * BW:(bx0 + XGRP) * BW],
                            stage[:].rearrange("p (dy x) -> p dy x", dy=BH),
                        )
    nc.compile()
    return nc


def _run(nc, in_maps, n_cores):
    from concourse.bass_utils import run_bass_kernel_spmd
    return run_bass_kernel_spmd(nc, in_maps, core_ids=list(range(n_cores)))


def kernel(hough_feat: np.ndarray) -> np.ndarray:
    hough_feat = np.asarray(hough_feat)
    assert hough_feat.shape == (N, C, NUMANGLE, NUMRHO)
    _, e_stream = _schedule()
    nc = build_bass(reps=1)
    in_maps = []
    for i in range(N):
        in_maps.append({
            "hp": _pack_h(hough_feat[i].astype(np.float32)),
            "e": e_stream,
        })
    res = _run(nc, in_maps, N)
    out = np.stack([r["out"].reshape(C, HIMG, WIMG) for r in res.results])
    return out.astype(hough_feat.dtype, copy=False)


if __name__ == "__main__":
    blocks, e_stream = _schedule()
    tc = sum(len(b) for b in blocks)
    print(f"chunks total={tc} avg/blk={tc/len(blocks):.1f} "
          f"max/blk={max(len(b) for b in blocks)} "
          f"E bytes={e_stream.nbytes/1e6:.1f} MB")

